# revision 11
# baseline (speedup 1.0000x reference)
"""Multi-head attention Bass kernel for Trainium2, 8 NeuronCores.

Problem: B=16, S=1024, D=256, H=16, DK=16.
  q = (Q@W_q + b_q)/sqrt(DK) per head; scores = exp(q.k); masked over key
  axis; attn = scores/(row_sum + 1e-8); context = attn@v.
Returns (output [B,S,D], attn_w [B,H,S,S]) like the reference.

Sharding: data-parallel over batch, 2 batches per core, weights replicated.

Per-core dataflow (all matmuls float32r, 1 cycle/row):
  - load Q/K/V naturally, PE-transpose to [D,S] layout
  - projection matmuls write q/k heads directly into 32-row "slots"
    (4 heads per 128-partition tile) with an augmentation row per slot:
    A = [q_hat_h ; 1], B = [k_h ; ln(mask)] so a single K=17 matmul
    produces scores + mask-fold in one shot, in either orientation.
  - scoresT pass: [sk,sq] tiles -> ACT exp -> maskedT (f32r)
  - context: vtil^T @ maskedT accumulated over sk, with a ones column in
    vtil so row 16 of the result is the row-sum of masked scores.
  - transpose context back, reciprocal of sums -> recip per query row
  - natural-layout attn tiles produced by PE-transposing maskedT tiles,
    normalized by recip during PSUM->SBUF eviction, DMA'd out.
"""

import os

os.environ.setdefault("JAX_PLATFORMS", "axon")

import numpy as np

B, S, D = 16, 1024, 256
H, DK = 16, 16
NCORES = 8
BSH = B // NCORES  # batches per core

_module_cache = {}


def _build_module():
    import concourse.bass as bass  # noqa: F401
    import concourse.mybir as mybir
    import concourse.tile as tile
    from concourse import bacc
    from concourse.masks import make_identity

    F32 = mybir.dt.float32
    F32R = mybir.dt.float32r
    I32 = mybir.dt.int32
    AF = mybir.ActivationFunctionType

    NEG = 30000.0  # exp(x - NEG) == 0.0 exactly for |x| < ~100

    nc = bacc.Bacc(None, target_bir_lowering=False)

    q_d = nc.dram_tensor("Q", [BSH, S, D], F32, kind="ExternalInput")
    k_d = nc.dram_tensor("K", [BSH, S, D], F32, kind="ExternalInput")
    v_d = nc.dram_tensor("V", [BSH, S, D], F32, kind="ExternalInput")
    m_d = nc.dram_tensor("mask", [BSH, S], I32, kind="ExternalInput")
    wq_d = nc.dram_tensor("W_q", [D, D], F32, kind="ExternalInput")
    bq_d = nc.dram_tensor("b_q", [D], F32, kind="ExternalInput")
    wk_d = nc.dram_tensor("W_k", [D, D], F32, kind="ExternalInput")
    wv_d = nc.dram_tensor("W_v", [D, D], F32, kind="ExternalInput")
    bv_d = nc.dram_tensor("b_v", [D], F32, kind="ExternalInput")

    out_d = nc.dram_tensor("out", [BSH, S, D], F32, kind="ExternalOutput")
    attn_d = nc.dram_tensor("attn", [BSH, H, S, S], F32, kind="ExternalOutput")

    NJ = S // 128  # 8 sequence tiles of 128
    NT = 4         # slot tiles (4 heads each)

    with tile.TileContext(nc) as tc:
        with tc.tile_pool(name="const", bufs=1) as const, \
             tc.tile_pool(name="nat", bufs=2) as natp, \
             tc.tile_pool(name="tT", bufs=6) as tTp, \
             tc.tile_pool(name="ab", bufs=1) as abp, \
             tc.tile_pool(name="vt", bufs=1) as vtp, \
             tc.tile_pool(name="mT", bufs=8) as mTp, \
             tc.tile_pool(name="ctx", bufs=2) as ctxp, \
             tc.tile_pool(name="rec", bufs=3) as recp, \
             tc.tile_pool(name="attn_st", bufs=3) as attp, \
             tc.tile_pool(name="outb", bufs=1) as outbp, \
             tc.tile_pool(name="rows", bufs=1) as rowsp, \
             tc.tile_pool(name="ps_big", bufs=2, space="PSUM") as psb, \
             tc.tile_pool(name="ps_ctx", bufs=2, space="PSUM") as psc:

            # ---------------- constants ----------------
            ident_f = const.tile([128, 128], F32)
            make_identity(nc, ident_f)
            ident_r = const.tile([128, 128], F32R)
            nc.vector.tensor_copy(out=ident_r, in_=ident_f)

            # W slot tiles: [128(din chunk), c, t, 128(slot col)]
            wq_s = const.tile([128, 2, NT, 128], F32)
            wk_s = const.tile([128, 2, NT, 128], F32)
            for w_sb, w_dram in ((wq_s, wq_d), (wk_s, wk_d)):
                for c in range(2):
                    for t in range(NT):
                        dst = w_sb[:, c, t, :].rearrange(
                            "p (g r) -> p g r", r=32)[:, :, 0:16]
                        src = w_dram.ap()[128 * c:128 * c + 128,
                                          64 * t:64 * t + 64].rearrange(
                            "p (g r) -> p g r", r=16)
                        nc.sync.dma_start(out=dst, in_=src)
                        pad = w_sb[:, c, t, :].rearrange(
                            "p (g r) -> p g r", r=32)[:, :, 16:32]
                        nc.vector.memset(pad, 0.0)

            wv_s = const.tile([128, 2, D], F32)
            nc.sync.dma_start(
                out=wv_s, in_=wv_d.ap().rearrange("(c p) d -> p c d", p=128))

            # b_q arranged into slot rows, one column per slot tile, /4
            bq_s = const.tile([128, NT], F32)
            nc.vector.memset(bq_s, 0.0)
            for t in range(NT):
                dst = bq_s.rearrange("(g x) t -> g x t", x=32)[:, 0:16, t:t + 1]
                src = bq_d.ap().rearrange(
                    "(t g r) -> t g r", t=NT, g=4)[t].unsqueeze(-1)
                nc.sync.dma_start(out=dst, in_=src)
            nc.scalar.mul(out=bq_s, in_=bq_s, mul=0.25)

            # b_v broadcast to all partitions
            bv_row = const.tile([1, D], F32)
            nc.sync.dma_start(out=bv_row, in_=bv_d.ap().unsqueeze(0))
            bv_b = const.tile([128, D], F32)
            nc.gpsimd.partition_broadcast(bv_b, bv_row)

            # aug helper: 1.0 at slot cols {16,48,80,112}
            aug_c = const.tile([1, 128], F32)
            nc.vector.memset(aug_c, 0.0)
            nc.vector.memset(
                aug_c.rearrange("o (g r) -> o g r", r=32)[:, :, 16], 1.0)
            onesA = const.tile([1, S], F32)
            nc.vector.memset(onesA, 4.0)  # x0.25 eviction scale -> 1.0
            ones16 = const.tile([128, 16], F32)
            nc.vector.memset(ones16, 1.0)

            for b in range(BSH):
                # ---------------- load + transpose inputs ----------------
                tposed = {}
                for name, dram in (("q", q_d), ("k", k_d), ("v", v_d)):
                    natt = natp.tile([128, NJ, D], F32, tag="nat")
                    nc.sync.dma_start(
                        out=natt,
                        in_=dram.ap()[b].rearrange("(j p) d -> p j d", p=128))
                    for c in range(2):
                        tt = tTp.tile([128, S], F32, tag="tT")
                        for jg in range(2):
                            pst = psc.tile([128, 512], F32, tag="ps_ctx")
                            for jj in range(4):
                                j = 4 * jg + jj
                                nc.tensor.transpose(
                                    pst[:, 128 * jj:128 * jj + 128],
                                    natt[:, j, 128 * c:128 * c + 128],
                                    ident_f)
                            nc.vector.tensor_copy(
                                out=tt[:, 512 * jg:512 * jg + 512], in_=pst)
                        tposed[(name, c)] = tt

                # mask -> lnm row
                m_f = rowsp.tile([1, S], F32, tag="mrow")
                nc.gpsimd.dma_start(out=m_f, in_=m_d.ap()[b:b + 1, :])
                lnm = rowsp.tile([1, S], F32, tag="lnm")
                nc.scalar.activation(out=lnm, in_=m_f, func=AF.Copy,
                                     scale=NEG, bias=-NEG)

                # ---------------- A / B slot tiles ----------------
                a_t = abp.tile([128, NT, S], F32R, tag="A")
                b_t = abp.tile([128, NT, S], F32R, tag="B")
                for (dst, w_sb, qt_name, aug_row, scale, bias) in (
                        (a_t, wq_s, "q", onesA, 0.25, bq_s),
                        (b_t, wk_s, "k", lnm, 1.0, None)):
                    for t in range(NT):
                        for sc in range(2):
                            ps = psc.tile([128, 512], F32, tag="ps_ctx")
                            for c in range(2):
                                nc.tensor.matmul(
                                    ps, w_sb[:, c, t, :],
                                    tposed[(qt_name, c)][:, 512 * sc:512 * sc + 512],
                                    start=(c == 0), stop=False)
                            nc.tensor.matmul(
                                ps, aug_c, aug_row[:, 512 * sc:512 * sc + 512],
                                start=False, stop=True)
                            if bias is not None:
                                nc.scalar.activation(
                                    out=dst[:, t, 512 * sc:512 * sc + 512],
                                    in_=ps, func=AF.Identity, scale=scale,
                                    bias=bias[:, t:t + 1])
                            else:
                                nc.scalar.activation(
                                    out=dst[:, t, 512 * sc:512 * sc + 512],
                                    in_=ps, func=AF.Copy, scale=scale)

                # ---------------- vtil: v heads + ones column ----------------
                # head h occupies cols 17h..17h+17 (v dims + ones col);
                # tail padded so a 32-wide lhsT slice is always in bounds.
                vtil = vtp.tile([128, NJ, 17 * H + 16], F32R, tag="vt")
                for j in range(NJ):
                    ps = psc.tile([128, 512], F32, tag="ps_ctx")
                    for c in range(2):
                        nc.tensor.matmul(
                            ps[:, 0:D],
                            tposed[("v", c)][:, 128 * j:128 * j + 128],
                            wv_s[:, c, :], start=(c == 0), stop=(c == 1))
                    hv = vtil[:, j, 0:17 * H].rearrange(
                        "p (h x) -> p h x", x=17)
                    nc.vector.tensor_add(
                        out=hv[:, :, 0:16],
                        in0=ps[:, 0:D].rearrange("p (h r) -> p h r", r=16),
                        in1=bv_b.rearrange("p (h r) -> p h r", r=16))
                    nc.vector.tensor_copy(out=hv[:, :, 16], in_=ones16)

                # ---------------- per head-pair ----------------
                # Heads are processed two at a time, in adjacent 32-row
                # groups of the same slot tile: their matmuls target
                # different row groups + different PSUM banks, so the PE
                # runs them concurrently and overlaps LDWEIGHTS (keeps
                # the HAM clock-gate warm).
                outb = outbp.tile([128, NJ, D], F32, tag="outb")
                for t in range(NT):
                    for gp in range(2):
                        heads = (4 * t + 2 * gp, 4 * t + 2 * gp + 1)
                        gpos = (32 * (2 * gp), 32 * (2 * gp + 1))

                        # scoresT pass + context accumulation, interleaved
                        # f32r matmuls must target psum partition base 0,
                        # so each head gets its own [17,S] accumulator.
                        psC = [psc.tile([17, S], F32, tag="ps_ctx",
                                        name="psC0"),
                               psc.tile([17, S], F32, tag="ps_ctx",
                                        name="psC1")]
                        recips = [None, None]
                        for j in range(NJ):
                            psT = [psb.tile([128, S], F32, tag="ps_big",
                                            name="psT0"),
                                   psb.tile([128, S], F32, tag="ps_big",
                                            name="psT1")]
                            for sc in range(2):
                                for hh in range(2):
                                    lo = gpos[hh]
                                    nc.tensor.matmul(
                                        psT[hh][:, 512 * sc:512 * sc + 512],
                                        b_t[lo:lo + 17, t, 128 * j:128 * j + 128],
                                        a_t[lo:lo + 17, t, 512 * sc:512 * sc + 512],
                                        start=True, stop=True,
                                        tile_position=(lo, 0))
                            for hh in range(2):
                                mt = mTp.tile([128, S], F32R, tag="mT")
                                nc.scalar.activation(out=mt, in_=psT[hh],
                                                     func=AF.Exp)
                                for sc in range(2):
                                    nc.tensor.matmul(
                                        psC[hh][:, 512 * sc:512 * sc + 512],
                                        vtil[:, j,
                                             17 * heads[hh]:17 * heads[hh] + 17],
                                        mt[:, 512 * sc:512 * sc + 512],
                                        start=(j == 0), stop=(j == NJ - 1))

                        # context epilogue per head: transpose back, recip
                        for hh in range(2):
                            h = heads[hh]
                            ctxT = ctxp.tile([17, S], F32, tag="ctx")
                            nc.vector.tensor_copy(out=ctxT, in_=psC[hh])
                            psCT = psc.tile([128, 512], F32, tag="ps_ctx")
                            for j in range(NJ):
                                nc.tensor.transpose(
                                    psCT[:, 64 * j:64 * j + 17],
                                    ctxT[:, 128 * j:128 * j + 128],
                                    ident_f[0:17, 0:17])
                            ctxN = ctxp.tile([128, 512], F32, tag="ctxN")
                            nc.vector.tensor_copy(out=ctxN, in_=psCT)
                            rsum = recp.tile([128, NJ], F32, tag="rsum")
                            nc.vector.tensor_scalar_add(
                                out=rsum,
                                in0=ctxN.rearrange(
                                    "p (j x) -> p j x", x=64)[:, :, 16],
                                scalar1=1e-8)
                            recip = recp.tile([128, NJ], F32, tag="recip")
                            nc.vector.reciprocal(out=recip, in_=rsum)
                            recips[hh] = recip
                            for j in range(NJ):
                                nc.vector.tensor_scalar_mul(
                                    out=outb[:, j, DK * h:DK * h + DK],
                                    in0=ctxN[:, 64 * j:64 * j + 16],
                                    scalar1=recip[:, j:j + 1])

                        # natural pass: recompute scores, exp, normalize, DMA
                        for j in range(NJ):
                            psN = [psb.tile([128, S], F32, tag="ps_big",
                                            name="psN0"),
                                   psb.tile([128, S], F32, tag="ps_big",
                                            name="psN1")]
                            for sc in range(2):
                                for hh in range(2):
                                    lo = gpos[hh]
                                    nc.tensor.matmul(
                                        psN[hh][:, 512 * sc:512 * sc + 512],
                                        a_t[lo:lo + 17, t, 128 * j:128 * j + 128],
                                        b_t[lo:lo + 17, t, 512 * sc:512 * sc + 512],
                                        start=True, stop=True,
                                        tile_position=(lo, 0))
                            for hh in range(2):
                                att = attp.tile([128, S], F32, tag="attn_st")
                                nc.scalar.activation(out=att, in_=psN[hh],
                                                     func=AF.Exp)
                                nc.vector.tensor_scalar_mul(
                                    out=att, in0=att,
                                    scalar1=recips[hh][:, j:j + 1])
                                nc.sync.dma_start(
                                    out=attn_d.ap()[b, heads[hh],
                                                    128 * j:128 * j + 128, :],
                                    in_=att)

                nc.sync.dma_start(
                    out=out_d.ap()[b].rearrange("(j p) d -> p j d", p=128),
                    in_=outb)

    nc.compile()
    return nc


def _get_module():
    if "nc" not in _module_cache:
        _module_cache["nc"] = _build_module()
    return _module_cache["nc"]


def kernel(Q, K, V, mask, W_q, b_q, W_k, W_v, b_v):
    from concourse.bass_utils import run_bass_kernel_spmd

    Q = np.ascontiguousarray(np.asarray(Q, dtype=np.float32))
    K = np.ascontiguousarray(np.asarray(K, dtype=np.float32))
    V = np.ascontiguousarray(np.asarray(V, dtype=np.float32))
    mask = np.ascontiguousarray(np.asarray(mask, dtype=np.int32))
    W_q = np.ascontiguousarray(np.asarray(W_q, dtype=np.float32))
    b_q = np.ascontiguousarray(np.asarray(b_q, dtype=np.float32))
    W_k = np.ascontiguousarray(np.asarray(W_k, dtype=np.float32))
    W_v = np.ascontiguousarray(np.asarray(W_v, dtype=np.float32))
    b_v = np.ascontiguousarray(np.asarray(b_v, dtype=np.float32))

    nc = _get_module()
    in_maps = []
    for i in range(NCORES):
        sl = slice(BSH * i, BSH * (i + 1))
        in_maps.append(dict(
            Q=Q[sl], K=K[sl], V=V[sl], mask=mask[sl],
            W_q=W_q, b_q=b_q, W_k=W_k, W_v=W_v, b_v=b_v))

    res = run_bass_kernel_spmd(nc, in_maps, core_ids=list(range(NCORES)))
    out = np.concatenate([r["out"] for r in res.results], axis=0)
    attn = np.concatenate([r["attn"] for r in res.results], axis=0)
    return out, attn


# revision 16
# speedup vs baseline: 1.0710x; 1.0710x over previous
"""Multi-head attention Bass kernel for Trainium2, 8 NeuronCores.

Problem: B=16, S=1024, D=256, H=16, DK=16.
  q = (Q@W_q + b_q)/sqrt(DK) per head; scores = exp(q.k); masked over key
  axis; attn = scores/(row_sum + 1e-8); context = attn@v.
Returns (output [B,S,D], attn_w [B,H,S,S]) like the reference.

Sharding: data-parallel over batch, 2 batches per core, weights replicated.

Per-core dataflow (all matmuls float32r, 1 cycle/row):
  - load Q/K/V naturally, PE-transpose to [D,S] layout
  - projection matmuls write q/k heads directly into 32-row "slots"
    (4 heads per 128-partition tile) with an augmentation row per slot:
    A = [q_hat_h ; 1], B = [k_h ; ln(mask)] so a single K=17 matmul
    produces scores + mask-fold in one shot, in either orientation.
  - scoresT pass: [sk,sq] tiles -> ACT exp -> maskedT (f32r)
  - context: vtil^T @ maskedT accumulated over sk, with a ones column in
    vtil so row 16 of the result is the row-sum of masked scores.
  - transpose context back, reciprocal of sums -> recip per query row
  - natural-layout attn tiles produced by PE-transposing maskedT tiles,
    normalized by recip during PSUM->SBUF eviction, DMA'd out.
"""

import os

os.environ.setdefault("JAX_PLATFORMS", "axon")

import numpy as np

B, S, D = 16, 1024, 256
H, DK = 16, 16
NCORES = 8
BSH = B // NCORES  # batches per core

_module_cache = {}


def _build_module():
    import concourse.bass as bass  # noqa: F401
    import concourse.mybir as mybir
    import concourse.tile as tile
    from concourse import bacc
    from concourse.masks import make_identity

    F32 = mybir.dt.float32
    F32R = mybir.dt.float32r
    BF16 = mybir.dt.bfloat16
    I32 = mybir.dt.int32
    AF = mybir.ActivationFunctionType

    NEG = 30000.0  # exp(x - NEG) == 0.0 exactly for |x| < ~100

    nc = bacc.Bacc(None, target_bir_lowering=False)

    q_d = nc.dram_tensor("Q", [BSH, S, D], F32, kind="ExternalInput")
    k_d = nc.dram_tensor("K", [BSH, S, D], F32, kind="ExternalInput")
    v_d = nc.dram_tensor("V", [BSH, S, D], F32, kind="ExternalInput")
    m_d = nc.dram_tensor("mask", [BSH, S], I32, kind="ExternalInput")
    wq_d = nc.dram_tensor("W_q", [D, D], F32, kind="ExternalInput")
    bq_d = nc.dram_tensor("b_q", [D], F32, kind="ExternalInput")
    wk_d = nc.dram_tensor("W_k", [D, D], F32, kind="ExternalInput")
    wv_d = nc.dram_tensor("W_v", [D, D], F32, kind="ExternalInput")
    bv_d = nc.dram_tensor("b_v", [D], F32, kind="ExternalInput")

    out_d = nc.dram_tensor("out", [BSH, S, D], F32, kind="ExternalOutput")
    attn_d = nc.dram_tensor("attn", [BSH, H, S, S], F32, kind="ExternalOutput")

    NJ = S // 128  # 8 sequence tiles of 128
    NT = 4         # slot tiles (4 heads each)

    with tile.TileContext(nc) as tc:
        with tc.tile_pool(name="const", bufs=1) as const, \
             tc.tile_pool(name="nat", bufs=2) as natp, \
             tc.tile_pool(name="tT", bufs=6) as tTp, \
             tc.tile_pool(name="ab", bufs=1) as abp, \
             tc.tile_pool(name="qn", bufs=3) as qnp, \
             tc.tile_pool(name="split", bufs=9) as splp, \
             tc.tile_pool(name="vt", bufs=1) as vtp, \
             tc.tile_pool(name="mT", bufs=8) as mTp, \
             tc.tile_pool(name="ctx", bufs=2) as ctxp, \
             tc.tile_pool(name="rec", bufs=3) as recp, \
             tc.tile_pool(name="attn_st", bufs=3) as attp, \
             tc.tile_pool(name="outb", bufs=1) as outbp, \
             tc.tile_pool(name="rows", bufs=1) as rowsp, \
             tc.tile_pool(name="ps_big", bufs=2, space="PSUM") as psb, \
             tc.tile_pool(name="ps_ctx", bufs=2, space="PSUM") as psc:

            # ---------------- constants ----------------
            ident_f = const.tile([128, 128], F32)
            make_identity(nc, ident_f)
            ident_b = const.tile([128, 128], BF16)
            nc.vector.tensor_copy(out=ident_b, in_=ident_f)

            # natural-layout weights [din(p), chunk, dout]; W_q pre-scaled 1/4
            wq_s = const.tile([128, 2, D], F32)
            nc.sync.dma_start(
                out=wq_s, in_=wq_d.ap().rearrange("(c p) d -> p c d", p=128))
            nc.scalar.mul(out=wq_s, in_=wq_s, mul=0.25)
            wk_s = const.tile([128, 2, D], F32)
            nc.sync.dma_start(
                out=wk_s, in_=wk_d.ap().rearrange("(c p) d -> p c d", p=128))
            wv_s = const.tile([128, 2, D], F32)
            nc.sync.dma_start(
                out=wv_s, in_=wv_d.ap().rearrange("(c p) d -> p c d", p=128))

            # biases broadcast to all partitions (b_q pre-scaled by 1/4)
            bq_row = const.tile([1, D], F32)
            nc.sync.dma_start(out=bq_row, in_=bq_d.ap().unsqueeze(0))
            bq_b = const.tile([128, D], F32)
            nc.gpsimd.partition_broadcast(bq_b, bq_row)
            nc.scalar.mul(out=bq_b, in_=bq_b, mul=0.25)
            bv_row = const.tile([1, D], F32)
            nc.sync.dma_start(out=bv_row, in_=bv_d.ap().unsqueeze(0))
            bv_b = const.tile([128, D], F32)
            nc.gpsimd.partition_broadcast(bv_b, bv_row)

            ones16 = const.tile([128, 16], F32)
            nc.vector.memset(ones16, 1.0)
            onescol = const.tile([128, 1], F32)
            nc.vector.memset(onescol, 1.0)

            for b in range(BSH):
                # ---------------- load + transpose inputs ----------------
                tposed = {}
                for name, dram in (("q", q_d), ("k", k_d), ("v", v_d)):
                    natt = natp.tile([128, NJ, D], F32, tag="nat")
                    nc.sync.dma_start(
                        out=natt,
                        in_=dram.ap()[b].rearrange("(j p) d -> p j d", p=128))
                    for c in range(2):
                        tt = tTp.tile([128, S], F32, tag="tT")
                        for jg in range(2):
                            pst = psc.tile([128, 512], F32, tag="ps_ctx")
                            for jj in range(4):
                                j = 4 * jg + jj
                                nc.tensor.transpose(
                                    pst[:, 128 * jj:128 * jj + 128],
                                    natt[:, j, 128 * c:128 * c + 128],
                                    ident_f)
                            nc.vector.tensor_copy(
                                out=tt[:, 512 * jg:512 * jg + 512], in_=pst)
                        tposed[(name, c)] = tt

                # mask -> lnm columns [128, NJ] (per sk tile)
                m_c = rowsp.tile([128, NJ], F32, tag="mrow")
                nc.gpsimd.dma_start(
                    out=m_c, in_=m_d.ap()[b].rearrange("(j p) -> p j", p=128))
                lnm_c = rowsp.tile([128, NJ], F32, tag="lnm")
                nc.scalar.activation(out=lnm_c, in_=m_c, func=AF.Copy,
                                     scale=NEG, bias=-NEG)

                # ------- A / B stacked bf16 hi/lo tiles -------
                # Per head a 64-row slot (2 heads per 128-row tile slice):
                # rows 0-15 hi, 16 aug (ones / ln-mask), 17-32 lo,
                # 33-48 hi again, 49-63 pad.  A single K=49 bf16 matmul
                # then computes hi.khi + aug + lo.khi + hi.klo.
                a_t = abp.tile([128, 8, S], BF16, tag="A")
                b_t = abp.tile([128, 8, S], BF16, tag="B")
                for (dst, w_sb, qt_name, bias) in (
                        (a_t, wq_s, "q", bq_b), (b_t, wk_s, "k", None)):
                    splits = []
                    for j in range(NJ):
                        psp = psc.tile([128, 256], F32, tag="ps_ctx",
                                       name="psproj")
                        for c in range(2):
                            nc.tensor.matmul(
                                psp, tposed[(qt_name, c)][:, 128 * j:128 * j + 128],
                                w_sb[:, c, :], start=(c == 0), stop=(c == 1))
                        qn = qnp.tile([128, D], F32, tag="qn")
                        if bias is not None:
                            nc.vector.tensor_add(out=qn, in0=psp, in1=bias)
                        else:
                            nc.scalar.copy(out=qn, in_=psp)
                        # natural split tile: per head a 64-col block
                        # A rows must pair as hi.khi + 1.lnm + lo.khi
                        # + hi.klo, so A = [hi, 1, lo, hi] while
                        # B = [khi, lnm, khi, klo].
                        sp = splp.tile([128, H, 64], BF16, tag="split")
                        qn3 = qn.rearrange("p (h r) -> p h r", r=16)
                        nc.vector.tensor_copy(out=sp[:, :, 0:16], in_=qn3)
                        if qt_name == "q":
                            nc.vector.tensor_sub(
                                out=sp[:, :, 17:33], in0=qn3,
                                in1=sp[:, :, 0:16])
                            nc.scalar.copy(out=sp[:, :, 33:49], in_=qn3)
                            nc.vector.memset(sp[:, :, 16:17], 1.0)
                        else:
                            nc.scalar.copy(out=sp[:, :, 17:33], in_=qn3)
                            nc.vector.tensor_sub(
                                out=sp[:, :, 33:49], in0=qn3,
                                in1=sp[:, :, 0:16])
                            nc.vector.tensor_scalar_mul(
                                out=sp[:, :, 16:17],
                                in0=ones16.unsqueeze(-1),
                                scalar1=lnm_c[:, j:j + 1])
                        splits.append(sp)
                    # transpose 128-col head-pair blocks into stacked rows
                    for ti in range(8):
                        for jg in range(2):
                            pst = psc.tile([128, 512], BF16, tag="ps_ctx",
                                           name="pstb")
                            for jj in range(4):
                                nc.tensor.transpose(
                                    pst[:, 128 * jj:128 * jj + 128],
                                    splits[4 * jg + jj][:, 2 * ti:2 * ti + 2, :]
                                    .rearrange("p h r -> p (h r)"),
                                    ident_b)
                            nc.vector.tensor_copy(
                                out=dst[:, ti, 512 * jg:512 * jg + 512],
                                in_=pst)

                # ---------------- vtil: v heads + ones column ----------------
                # head h occupies cols 17h..17h+17 (v dims + ones col);
                # tail padded so a 32-wide lhsT slice is always in bounds.
                vtil = vtp.tile([128, NJ, 17 * H + 16], F32R, tag="vt")
                for j in range(NJ):
                    ps = psc.tile([128, 512], F32, tag="ps_ctx")
                    for c in range(2):
                        nc.tensor.matmul(
                            ps[:, 0:D],
                            tposed[("v", c)][:, 128 * j:128 * j + 128],
                            wv_s[:, c, :], start=(c == 0), stop=(c == 1))
                    hv = vtil[:, j, 0:17 * H].rearrange(
                        "p (h x) -> p h x", x=17)
                    nc.vector.tensor_add(
                        out=hv[:, :, 0:16],
                        in0=ps[:, 0:D].rearrange("p (h r) -> p h r", r=16),
                        in1=bv_b.rearrange("p (h r) -> p h r", r=16))
                    nc.vector.tensor_copy(out=hv[:, :, 16], in_=ones16)

                # ---------------- per head-pair ----------------
                # Heads are processed two at a time, in adjacent 32-row
                # groups of the same slot tile: their matmuls target
                # different row groups + different PSUM banks, so the PE
                # runs them concurrently and overlaps LDWEIGHTS (keeps
                # the HAM clock-gate warm).
                outb = outbp.tile([128, NJ, D], F32, tag="outb")
                for ti in range(8):
                    if True:
                        heads = (2 * ti, 2 * ti + 1)
                        gpos = (0, 64)

                        # scoresT pass + context accumulation, interleaved
                        # f32r matmuls must target psum partition base 0,
                        # so each head gets its own [17,S] accumulator.
                        psC = [psc.tile([17, S], F32, tag="ps_ctx",
                                        name="psC0"),
                               psc.tile([17, S], F32, tag="ps_ctx",
                                        name="psC1")]
                        recips = [None, None]
                        for j in range(NJ):
                            psT = [psb.tile([128, S], F32, tag="ps_big",
                                            name="psT0"),
                                   psb.tile([128, S], F32, tag="ps_big",
                                            name="psT1")]
                            for sc in range(2):
                                for hh in range(2):
                                    lo = gpos[hh]
                                    nc.tensor.matmul(
                                        psT[hh][:, 512 * sc:512 * sc + 512],
                                        b_t[lo:lo + 49, ti, 128 * j:128 * j + 128],
                                        a_t[lo:lo + 49, ti, 512 * sc:512 * sc + 512],
                                        start=True, stop=True,
                                        tile_position=(lo, 0))
                            for hh in range(2):
                                mt = mTp.tile([128, S], F32R, tag="mT")
                                nc.scalar.activation(out=mt, in_=psT[hh],
                                                     func=AF.Exp)
                                for sc in range(2):
                                    nc.tensor.matmul(
                                        psC[hh][:, 512 * sc:512 * sc + 512],
                                        vtil[:, j,
                                             17 * heads[hh]:17 * heads[hh] + 17],
                                        mt[:, 512 * sc:512 * sc + 512],
                                        start=(j == 0), stop=(j == NJ - 1))

                        # context epilogue per head: transpose back, recip
                        for hh in range(2):
                            h = heads[hh]
                            ctxT = ctxp.tile([17, S], F32, tag="ctx")
                            nc.vector.tensor_copy(out=ctxT, in_=psC[hh])
                            psCT = psc.tile([128, 512], F32, tag="ps_ctx")
                            for j in range(NJ):
                                nc.tensor.transpose(
                                    psCT[:, 64 * j:64 * j + 17],
                                    ctxT[:, 128 * j:128 * j + 128],
                                    ident_f[0:17, 0:17])
                            ctxN = ctxp.tile([128, 512], F32, tag="ctxN")
                            nc.vector.tensor_copy(out=ctxN, in_=psCT)
                            rsum = recp.tile([128, NJ], F32, tag="rsum")
                            nc.vector.tensor_scalar_add(
                                out=rsum,
                                in0=ctxN.rearrange(
                                    "p (j x) -> p j x", x=64)[:, :, 16],
                                scalar1=1e-8)
                            recip = recp.tile([128, NJ], F32, tag="recip")
                            nc.vector.reciprocal(out=recip, in_=rsum)
                            recips[hh] = recip
                            for j in range(NJ):
                                nc.vector.tensor_scalar_mul(
                                    out=outb[:, j, DK * h:DK * h + DK],
                                    in0=ctxN[:, 64 * j:64 * j + 16],
                                    scalar1=recip[:, j:j + 1])

                        # natural pass: recompute scores, exp, normalize, DMA
                        for j in range(NJ):
                            psN = [psb.tile([128, S], F32, tag="ps_big",
                                            name="psN0"),
                                   psb.tile([128, S], F32, tag="ps_big",
                                            name="psN1")]
                            for sc in range(2):
                                for hh in range(2):
                                    lo = gpos[hh]
                                    nc.tensor.matmul(
                                        psN[hh][:, 512 * sc:512 * sc + 512],
                                        a_t[lo:lo + 49, ti, 128 * j:128 * j + 128],
                                        b_t[lo:lo + 49, ti, 512 * sc:512 * sc + 512],
                                        start=True, stop=True,
                                        tile_position=(lo, 0))
                            for hh in range(2):
                                att = attp.tile([128, S], F32, tag="attn_st")
                                nc.scalar.activation(out=att, in_=psN[hh],
                                                     func=AF.Exp)
                                nc.vector.tensor_scalar_mul(
                                    out=att, in0=att,
                                    scalar1=recips[hh][:, j:j + 1])
                                nc.sync.dma_start(
                                    out=attn_d.ap()[b, heads[hh],
                                                    128 * j:128 * j + 128, :],
                                    in_=att)

                nc.sync.dma_start(
                    out=out_d.ap()[b].rearrange("(j p) d -> p j d", p=128),
                    in_=outb)

    nc.compile()
    return nc


def _get_module():
    if "nc" not in _module_cache:
        _module_cache["nc"] = _build_module()
    return _module_cache["nc"]


def kernel(Q, K, V, mask, W_q, b_q, W_k, W_v, b_v):
    from concourse.bass_utils import run_bass_kernel_spmd

    Q = np.ascontiguousarray(np.asarray(Q, dtype=np.float32))
    K = np.ascontiguousarray(np.asarray(K, dtype=np.float32))
    V = np.ascontiguousarray(np.asarray(V, dtype=np.float32))
    mask = np.ascontiguousarray(np.asarray(mask, dtype=np.int32))
    W_q = np.ascontiguousarray(np.asarray(W_q, dtype=np.float32))
    b_q = np.ascontiguousarray(np.asarray(b_q, dtype=np.float32))
    W_k = np.ascontiguousarray(np.asarray(W_k, dtype=np.float32))
    W_v = np.ascontiguousarray(np.asarray(W_v, dtype=np.float32))
    b_v = np.ascontiguousarray(np.asarray(b_v, dtype=np.float32))

    nc = _get_module()
    in_maps = []
    for i in range(NCORES):
        sl = slice(BSH * i, BSH * (i + 1))
        in_maps.append(dict(
            Q=Q[sl], K=K[sl], V=V[sl], mask=mask[sl],
            W_q=W_q, b_q=b_q, W_k=W_k, W_v=W_v, b_v=b_v))

    res = run_bass_kernel_spmd(nc, in_maps, core_ids=list(range(NCORES)))
    out = np.concatenate([r["out"] for r in res.results], axis=0)
    attn = np.concatenate([r["attn"] for r in res.results], axis=0)
    return out, attn


# revision 18
# speedup vs baseline: 1.1698x; 1.0923x over previous
"""Multi-head attention Bass kernel for Trainium2, 8 NeuronCores.

Problem: B=16, S=1024, D=256, H=16, DK=16.
  q = (Q@W_q + b_q)/sqrt(DK) per head; scores = exp(q.k); masked over key
  axis; attn = scores/(row_sum + 1e-8); context = attn@v.
Returns (output [B,S,D], attn_w [B,H,S,S]) like the reference.

Sharding: data-parallel over batch, 2 batches per core, weights replicated.

Per-core dataflow (all matmuls float32r, 1 cycle/row):
  - load Q/K/V naturally, PE-transpose to [D,S] layout
  - projection matmuls write q/k heads directly into 32-row "slots"
    (4 heads per 128-partition tile) with an augmentation row per slot:
    A = [q_hat_h ; 1], B = [k_h ; ln(mask)] so a single K=17 matmul
    produces scores + mask-fold in one shot, in either orientation.
  - scoresT pass: [sk,sq] tiles -> ACT exp -> maskedT (f32r)
  - context: vtil^T @ maskedT accumulated over sk, with a ones column in
    vtil so row 16 of the result is the row-sum of masked scores.
  - transpose context back, reciprocal of sums -> recip per query row
  - natural-layout attn tiles produced by PE-transposing maskedT tiles,
    normalized by recip during PSUM->SBUF eviction, DMA'd out.
"""

import os

os.environ.setdefault("JAX_PLATFORMS", "axon")

import numpy as np

B, S, D = 16, 1024, 256
H, DK = 16, 16
NCORES = 8
BSH = B // NCORES  # batches per core

_module_cache = {}


def _build_module():
    import concourse.bass as bass  # noqa: F401
    import concourse.mybir as mybir
    import concourse.tile as tile
    from concourse import bacc
    from concourse.masks import make_identity

    F32 = mybir.dt.float32
    F32R = mybir.dt.float32r
    BF16 = mybir.dt.bfloat16
    I32 = mybir.dt.int32
    AF = mybir.ActivationFunctionType

    NEG = 30000.0  # exp(x - NEG) == 0.0 exactly for |x| < ~100

    nc = bacc.Bacc(None, target_bir_lowering=False)

    q_d = nc.dram_tensor("Q", [BSH, S, D], F32, kind="ExternalInput")
    k_d = nc.dram_tensor("K", [BSH, S, D], F32, kind="ExternalInput")
    v_d = nc.dram_tensor("V", [BSH, S, D], F32, kind="ExternalInput")
    m_d = nc.dram_tensor("mask", [BSH, S], I32, kind="ExternalInput")
    wq_d = nc.dram_tensor("W_q", [D, D], F32, kind="ExternalInput")
    bq_d = nc.dram_tensor("b_q", [D], F32, kind="ExternalInput")
    wk_d = nc.dram_tensor("W_k", [D, D], F32, kind="ExternalInput")
    wv_d = nc.dram_tensor("W_v", [D, D], F32, kind="ExternalInput")
    bv_d = nc.dram_tensor("b_v", [D], F32, kind="ExternalInput")

    out_d = nc.dram_tensor("out", [BSH, S, D], F32, kind="ExternalOutput")
    attn_d = nc.dram_tensor("attn", [BSH, H, S, S], F32, kind="ExternalOutput")

    NJ = S // 128  # 8 sequence tiles of 128
    NT = 4         # slot tiles (4 heads each)

    with tile.TileContext(nc) as tc:
        with tc.tile_pool(name="const", bufs=1) as const, \
             tc.tile_pool(name="nat", bufs=1) as natp, \
             tc.tile_pool(name="tT", bufs=2) as tTp, \
             tc.tile_pool(name="ab", bufs=1) as abp, \
             tc.tile_pool(name="qn", bufs=2) as qnp, \
             tc.tile_pool(name="split", bufs=3) as splp, \
             tc.tile_pool(name="vt", bufs=1) as vtp, \
             tc.tile_pool(name="mT", bufs=6) as mTp, \
             tc.tile_pool(name="ctx", bufs=2) as ctxp, \
             tc.tile_pool(name="rec", bufs=3) as recp, \
             tc.tile_pool(name="attn_st", bufs=4) as attp, \
             tc.tile_pool(name="outb", bufs=1) as outbp, \
             tc.tile_pool(name="rows", bufs=1) as rowsp, \
             tc.tile_pool(name="ps_big", bufs=2, space="PSUM") as psb, \
             tc.tile_pool(name="ps_ctx", bufs=2, space="PSUM") as psc:

            # ---------------- constants ----------------
            ident_f = const.tile([128, 128], F32)
            make_identity(nc, ident_f)
            ident_b = const.tile([128, 128], BF16)
            nc.vector.tensor_copy(out=ident_b, in_=ident_f)

            # natural-layout weights [din(p), chunk, dout]; W_q pre-scaled 1/4
            wq_s = const.tile([128, 2, D], F32)
            nc.sync.dma_start(
                out=wq_s, in_=wq_d.ap().rearrange("(c p) d -> p c d", p=128))
            nc.scalar.mul(out=wq_s, in_=wq_s, mul=0.25)
            wk_s = const.tile([128, 2, D], F32)
            nc.sync.dma_start(
                out=wk_s, in_=wk_d.ap().rearrange("(c p) d -> p c d", p=128))
            wv_s = const.tile([128, 2, D], F32)
            nc.sync.dma_start(
                out=wv_s, in_=wv_d.ap().rearrange("(c p) d -> p c d", p=128))

            # biases broadcast to all partitions (b_q pre-scaled by 1/4)
            bq_row = const.tile([1, D], F32)
            nc.sync.dma_start(out=bq_row, in_=bq_d.ap().unsqueeze(0))
            bq_b = const.tile([128, D], F32)
            nc.gpsimd.partition_broadcast(bq_b, bq_row)
            nc.scalar.mul(out=bq_b, in_=bq_b, mul=0.25)
            bv_row = const.tile([1, D], F32)
            nc.sync.dma_start(out=bv_row, in_=bv_d.ap().unsqueeze(0))
            bv_b = const.tile([128, D], F32)
            nc.gpsimd.partition_broadcast(bv_b, bv_row)

            ones16 = const.tile([128, 16], F32)
            nc.vector.memset(ones16, 1.0)
            onescol = const.tile([128, 1], F32)
            nc.vector.memset(onescol, 1.0)

            for b in range(BSH):
                # ---- load inputs, transpose, project, build bf16 stacks ----
                # Per head a 128-row slice: rows 0-15 hi, 16 aug, 17-32 lo,
                # 33-48 hi-dup (A) / swapped for B, 49-127 ZERO so every
                # scores matmul is K=128 (full-row: keeps the HAM clock
                # ungated).
                a_t = abp.tile([128, H, S], BF16, tag="A")
                b_t = abp.tile([128, H, S], BF16, tag="B")

                # mask -> lnm columns [128, NJ] (per sk tile)
                m_c = rowsp.tile([128, NJ], F32, tag="mrow")
                nc.gpsimd.dma_start(
                    out=m_c, in_=m_d.ap()[b].rearrange("(j p) -> p j", p=128))
                lnm_c = rowsp.tile([128, NJ], F32, tag="lnm")
                nc.scalar.activation(out=lnm_c, in_=m_c, func=AF.Copy,
                                     scale=NEG, bias=-NEG)

                tposed = {}
                for name, dram in (("q", q_d), ("k", k_d), ("v", v_d)):
                    natt = natp.tile([128, NJ, D], F32, tag="nat")
                    nc.sync.dma_start(
                        out=natt,
                        in_=dram.ap()[b].rearrange("(j p) d -> p j d", p=128))
                    tts = []
                    for c in range(2):
                        tt = tTp.tile([128, S], F32, tag="tT")
                        for jg in range(2):
                            pst = psc.tile([128, 512], F32, tag="ps_ctx",
                                           name="pstr")
                            for jj in range(4):
                                j = 4 * jg + jj
                                nc.tensor.transpose(
                                    pst[:, 128 * jj:128 * jj + 128],
                                    natt[:, j, 128 * c:128 * c + 128],
                                    ident_f)
                            nc.vector.tensor_copy(
                                out=tt[:, 512 * jg:512 * jg + 512], in_=pst)
                        tts.append(tt)

                    if name == "v":
                        tposed["v0"], tposed["v1"] = tts
                        break

                    dst = a_t if name == "q" else b_t
                    bias = bq_b if name == "q" else None
                    w_sb = wq_s if name == "q" else wk_s
                    for j in range(NJ):
                        psp = psc.tile([128, 256], F32, tag="ps_ctx",
                                       name="psproj")
                        for c in range(2):
                            nc.tensor.matmul(
                                psp, tts[c][:, 128 * j:128 * j + 128],
                                w_sb[:, c, :], start=(c == 0), stop=(c == 1))
                        qn = qnp.tile([128, D], F32, tag="qn")
                        if bias is not None:
                            nc.vector.tensor_add(out=qn, in0=psp, in1=bias)
                        else:
                            nc.scalar.copy(out=qn, in_=psp)
                        # natural split tile: per head a 128-col block
                        # (cols 49-127 zeroed)
                        sp = splp.tile([128, H, 128], BF16, tag="split")
                        qn3 = qn.rearrange("p (h r) -> p h r", r=16)
                        nc.vector.tensor_copy(out=sp[:, :, 0:16], in_=qn3)
                        if name == "q":
                            nc.vector.tensor_sub(
                                out=sp[:, :, 17:33], in0=qn3,
                                in1=sp[:, :, 0:16])
                            nc.scalar.copy(out=sp[:, :, 33:49], in_=qn3)
                            nc.vector.memset(sp[:, :, 16:17], 1.0)
                        else:
                            nc.scalar.copy(out=sp[:, :, 17:33], in_=qn3)
                            nc.vector.tensor_sub(
                                out=sp[:, :, 33:49], in0=qn3,
                                in1=sp[:, :, 0:16])
                            nc.vector.tensor_scalar_mul(
                                out=sp[:, :, 16:17],
                                in0=ones16.unsqueeze(-1),
                                scalar1=lnm_c[:, j:j + 1])
                        nc.vector.memset(sp[:, :, 49:128], 0.0)
                        # transpose each head's block into stacked K rows
                        for hg in range(2):
                            pst = psc.tile([128, 1024], BF16, tag="ps_ctx",
                                           name="pstb")
                            for hh in range(8):
                                h = 8 * hg + hh
                                nc.tensor.transpose(
                                    pst[:, 128 * hh:128 * hh + 128],
                                    sp[:, h, :], ident_b)
                            nc.vector.tensor_copy(
                                out=dst[:, 8 * hg:8 * hg + 8,
                                        128 * j:128 * j + 128],
                                in_=pst.rearrange("p (h x) -> p h x", x=128))

                # ---------------- vtil: v heads + ones column ----------------
                # head h occupies cols 17h..17h+17 (v dims + ones col);
                # tail padded so a 32-wide lhsT slice is always in bounds.
                vtil = vtp.tile([128, NJ, 17 * H + 16], F32R, tag="vt")
                for j in range(NJ):
                    ps = psc.tile([128, 512], F32, tag="ps_ctx")
                    for c in range(2):
                        nc.tensor.matmul(
                            ps[:, 0:D],
                            tposed[f"v{c}"][:, 128 * j:128 * j + 128],
                            wv_s[:, c, :], start=(c == 0), stop=(c == 1))
                    hv = vtil[:, j, 0:17 * H].rearrange(
                        "p (h x) -> p h x", x=17)
                    nc.vector.tensor_add(
                        out=hv[:, :, 0:16],
                        in0=ps[:, 0:D].rearrange("p (h r) -> p h r", r=16),
                        in1=bv_b.rearrange("p (h r) -> p h r", r=16))
                    nc.vector.tensor_copy(out=hv[:, :, 16], in_=ones16)

                # ---------------- per head-pair ----------------
                # Heads are processed two at a time, in adjacent 32-row
                # groups of the same slot tile: their matmuls target
                # different row groups + different PSUM banks, so the PE
                # runs them concurrently and overlaps LDWEIGHTS (keeps
                # the HAM clock-gate warm).
                outb = outbp.tile([128, NJ, D], F32, tag="outb")
                for h in range(H):
                    # scoresT pass + context accumulation, interleaved
                    psC = psc.tile([17, S], F32, tag="ps_ctx", name="psC")
                    for j in range(NJ):
                        psT = psb.tile([128, S], F32, tag="ps_big",
                                       name="psT")
                        for sc in range(2):
                            nc.tensor.matmul(
                                psT[:, 512 * sc:512 * sc + 512],
                                b_t[:, h, 128 * j:128 * j + 128],
                                a_t[:, h, 512 * sc:512 * sc + 512],
                                start=True, stop=True)
                        mt = mTp.tile([128, S], F32R, tag="mT")
                        nc.scalar.activation(out=mt, in_=psT, func=AF.Exp)
                        for sc in range(2):
                            nc.tensor.matmul(
                                psC[:, 512 * sc:512 * sc + 512],
                                vtil[:, j, 17 * h:17 * h + 17],
                                mt[:, 512 * sc:512 * sc + 512],
                                start=(j == 0), stop=(j == NJ - 1))

                    # context epilogue: transpose back, recip
                    ctxT = ctxp.tile([17, S], F32, tag="ctx")
                    nc.vector.tensor_copy(out=ctxT, in_=psC)
                    psCT = psc.tile([128, 512], F32, tag="ps_ctx",
                                    name="psCT")
                    for j in range(NJ):
                        nc.tensor.transpose(
                            psCT[:, 64 * j:64 * j + 17],
                            ctxT[:, 128 * j:128 * j + 128],
                            ident_f[0:17, 0:17])
                    ctxN = ctxp.tile([128, 512], F32, tag="ctxN")
                    nc.vector.tensor_copy(out=ctxN, in_=psCT)
                    rsum = recp.tile([128, NJ], F32, tag="rsum")
                    nc.vector.tensor_scalar_add(
                        out=rsum,
                        in0=ctxN.rearrange("p (j x) -> p j x", x=64)[:, :, 16],
                        scalar1=1e-8)
                    recip = recp.tile([128, NJ], F32, tag="recip")
                    nc.vector.reciprocal(out=recip, in_=rsum)
                    for j in range(NJ):
                        nc.vector.tensor_scalar_mul(
                            out=outb[:, j, DK * h:DK * h + DK],
                            in0=ctxN[:, 64 * j:64 * j + 16],
                            scalar1=recip[:, j:j + 1])

                    # natural pass: recompute scores, exp, normalize, DMA
                    for j in range(NJ):
                        psN = psb.tile([128, S], F32, tag="ps_big",
                                       name="psN")
                        for sc in range(2):
                            nc.tensor.matmul(
                                psN[:, 512 * sc:512 * sc + 512],
                                a_t[:, h, 128 * j:128 * j + 128],
                                b_t[:, h, 512 * sc:512 * sc + 512],
                                start=True, stop=True)
                        att = attp.tile([128, S], F32, tag="attn_st")
                        nc.scalar.activation(out=att, in_=psN, func=AF.Exp)
                        nc.vector.tensor_scalar_mul(
                            out=att, in0=att, scalar1=recip[:, j:j + 1])
                        nc.sync.dma_start(
                            out=attn_d.ap()[b, h, 128 * j:128 * j + 128, :],
                            in_=att)

                nc.sync.dma_start(
                    out=out_d.ap()[b].rearrange("(j p) d -> p j d", p=128),
                    in_=outb)

    nc.compile()
    return nc


def _get_module():
    if "nc" not in _module_cache:
        _module_cache["nc"] = _build_module()
    return _module_cache["nc"]


def kernel(Q, K, V, mask, W_q, b_q, W_k, W_v, b_v):
    from concourse.bass_utils import run_bass_kernel_spmd

    Q = np.ascontiguousarray(np.asarray(Q, dtype=np.float32))
    K = np.ascontiguousarray(np.asarray(K, dtype=np.float32))
    V = np.ascontiguousarray(np.asarray(V, dtype=np.float32))
    mask = np.ascontiguousarray(np.asarray(mask, dtype=np.int32))
    W_q = np.ascontiguousarray(np.asarray(W_q, dtype=np.float32))
    b_q = np.ascontiguousarray(np.asarray(b_q, dtype=np.float32))
    W_k = np.ascontiguousarray(np.asarray(W_k, dtype=np.float32))
    W_v = np.ascontiguousarray(np.asarray(W_v, dtype=np.float32))
    b_v = np.ascontiguousarray(np.asarray(b_v, dtype=np.float32))

    nc = _get_module()
    in_maps = []
    for i in range(NCORES):
        sl = slice(BSH * i, BSH * (i + 1))
        in_maps.append(dict(
            Q=Q[sl], K=K[sl], V=V[sl], mask=mask[sl],
            W_q=W_q, b_q=b_q, W_k=W_k, W_v=W_v, b_v=b_v))

    res = run_bass_kernel_spmd(nc, in_maps, core_ids=list(range(NCORES)))
    out = np.concatenate([r["out"] for r in res.results], axis=0)
    attn = np.concatenate([r["attn"] for r in res.results], axis=0)
    return out, attn
